# revision 11
# baseline (speedup 1.0000x reference)
"""Trainium2 Bass kernel for nn_BeliefField (belief-field recurrent PDE cell).

Data-parallel over batch: 64 samples -> 8 NeuronCores x 8 samples each.
State u kept channel-major [C=2048 partitions, (b h w)=2048 free] per core.
Hebbian batch-mean correlation is AllReduced across cores in bf16.
"""

import numpy as np

import concourse.bacc as bacc
import concourse.mybir as mybir
import concourse.tile as tile
from concourse.bass_utils import run_bass_kernel_spmd
from concourse.masks import make_identity

F32 = mybir.dt.float32
F32R = mybir.dt.float32r
BF16 = mybir.dt.bfloat16
AF = mybir.ActivationFunctionType
OP = mybir.AluOpType

B, C, HW, SD = 64, 2048, 16, 6
NSP = HW * HW                  # 256 spatial positions
NCORES = 8
BL = B // NCORES               # 8 local samples
R = BL * NSP                   # 2048 local rows
P = 128
NP = C // P                    # 16 channel ptiles
NB = BL                        # 8 row blocks (1 sample each)
BS = NSP                       # 256 cols per block
NRT = R // P                   # 16 row chunks of u_r
N_STEPS = 3
HEB_LR, HEB_DECAY = 0.01, 0.99
DIFF_SCALE = 0.25
DILS = (1, 4)

MM_DT = F32        # dtype of big matmul operands (F32 | F32R | BF16)
W_BUFS = 24        # weight-tile buffering (shared tag)
U_BUFS = 2

# vec rows (packed per-channel vectors)
(V_GATEB, V_VALB, V_R1B, V_R2B, V_RMSW, V_LNG, V_LNB, V_READB, V_S2B,
 V_CA, V_CB, V_CSN4, V_S1B) = range(13)
NVEC = 13


def declare_io(nc):
    din = {}

    def inp(name, shape):
        din[name] = nc.dram_tensor(name, list(shape), F32, kind="ExternalInput").ap()

    inp("state_loc", (BL, C, NSP))
    inp("obs_loc", (BL, C))
    inp("stim_loc", (BL, SD))
    inp("h_loc", (BL, C))
    inp("heb_W", (C, C))
    inp("gate_W", (C, C)); inp("value_W", (C, C))
    inp("r1_W", (C, C)); inp("r2_W", (C, C))
    inp("read_W", (C, C)); inp("s2_W", (C, C))
    inp("gru_Wih", (C, 3 * C)); inp("gru_Whh", (C, 3 * C))
    inp("s1_W", (SD, C))
    inp("vecs", (NVEC, C))
    inp("gvecs", (2, 3 * C))
    u_out = nc.dram_tensor("u_out", [BL, C, NSP], F32, kind="ExternalOutput").ap()
    rep_out = nc.dram_tensor("rep_out", [BL, C], F32, kind="ExternalOutput").ap()
    return din, u_out, rep_out


def emit(nc, tc, din, u_out, rep_out, sc):
    decay, dt, ap_sp = sc["decay"], sc["dt"], sc["ap_sp"]
    lr_eff = HEB_LR / float(B * NSP)

    dram = tc.alloc_tile_pool(name="dram", bufs=1, space="DRAM")
    sbp = tc.alloc_tile_pool(name="persist", bufs=1)
    sbw = tc.alloc_tile_pool(name="work", bufs=1)
    psp = tc.alloc_tile_pool(name="psum", bufs=1, space="PSUM")

    u_master = dram.tile([C, R], F32, name="u_master")
    wh_mm = dram.tile([C, C], F32, name="wh_mm")
    cc_in_l = [dram.tile([C, C], BF16, name=f"cc_in{t}") for t in range(N_STEPS)]
    cc_out_l = [dram.tile([C, C], BF16, name=f"cc_out{t}", addr_space="Shared")
                for t in range(N_STEPS)]

    # ---- persistent small tiles ----
    vt = sbp.tile([P, NVEC, NP], F32, name="vt")
    nc.sync.dma_start(vt[:], din["vecs"].rearrange("v (j p) -> p v j", p=P))
    gvt = sbp.tile([P, 2, 48], F32, name="gvt")
    nc.sync.dma_start(gvt[:], din["gvecs"].rearrange("v (j p) -> p v j", p=P))

    def pv(row, j):  # per-partition scalar AP [P,1]
        return vt[:, row, j:j + 1]

    ident = sbp.tile([P, P], F32, name="ident")
    make_identity(nc, ident[:])
    ones = sbp.tile([P, 1], F32, name="ones_col")
    nc.vector.memset(ones[:], 1.0)
    eps6 = sbp.tile([P, 1], F32, name="eps6")
    nc.vector.memset(eps6[:], 1e-6)
    eps5 = sbp.tile([P, 1], F32, name="eps5")
    nc.vector.memset(eps5[:], 1e-5)

    u_r = [sbp.tile([P, C], BF16, name=f"ur{i}") for i in range(NRT)]
    pooled = [sbp.tile([P, BL], F32, name=f"pool{j}") for j in range(NP)]
    h_t = [sbp.tile([P, BL], F32, name=f"h{j}") for j in range(NP)]
    stim_cm = [sbp.tile([P, BL], F32, name=f"stim{j}") for j in range(NP)]
    sbdt = [sbp.tile([P, BL], F32, name=f"sbdt{j}") for j in range(NP)]
    obs_cm = [sbp.tile([P, BL], F32, name=f"obs{j}") for j in range(NP)]
    gv_t = [sbp.tile([P, BL], F32, name=f"gv{j}") for j in range(NP)]

    for j in range(NP):
        nc.sync.dma_start(obs_cm[j][:], din["obs_loc"][:, j * P:(j + 1) * P].rearrange("b c -> c b"))
        nc.sync.dma_start(h_t[j][:], din["h_loc"][:, j * P:(j + 1) * P].rearrange("b c -> c b"))

    def stream_w(wdram, k, mj, tag, mw=P):
        t = sbw.tile([P, mw], F32, name=f"{tag}_{k}_{mj}", tag="wst", bufs=W_BUFS)
        nc.sync.dma_start(t[:], wdram[k * P:(k + 1) * P, mj * mw:(mj + 1) * mw])
        if MM_DT != F32:
            tr = sbw.tile([P, mw], MM_DT, name=f"{tag}r_{k}_{mj}", tag="wstr", bufs=W_BUFS)
            nc.vector.tensor_copy(tr[:], t[:])
            return tr[:]
        return t[:]

    # =========== write phase: u0 = decay*state + gate*value ===========
    for mj in range(NP):
        psg = psp.tile([P, BL], F32, name=f"psg{mj}", tag="sm", bufs=2)
        for k in range(NP):
            wt = stream_w(din["gate_W"], k, mj, "wg")
            nc.tensor.matmul(psg[:], wt, obs_cm[k][:], start=(k == 0), stop=(k == NP - 1))
        gt = sbw.tile([P, BL], F32, name=f"gt{mj}", tag="gt", bufs=2)
        nc.scalar.activation(gt[:], psg[:], AF.Sigmoid, bias=pv(V_GATEB, mj))
        psv = psp.tile([P, BL], F32, name=f"psv{mj}", tag="sm", bufs=2)
        for k in range(NP):
            wt = stream_w(din["value_W"], k, mj, "wv")
            nc.tensor.matmul(psv[:], wt, obs_cm[k][:], start=(k == 0), stop=(k == NP - 1))
        vt2 = sbw.tile([P, BL], F32, name=f"vt2{mj}", tag="vt2", bufs=2)
        nc.scalar.activation(vt2[:], psv[:], AF.Tanh, bias=pv(V_VALB, mj))
        nc.vector.tensor_tensor(gv_t[mj][:], gt[:], vt2[:], OP.mult)

    # stim = gelu(stimulus @ s1 + s1_b) @ s2 + s2_b  (channel-major [P, BL])
    stimT = sbp.tile([SD, BL], F32, name="stimT")
    nc.sync.dma_start(stimT[:], din["stim_loc"].rearrange("b s -> s b"))
    s1sb = sbp.tile([SD, C], F32, name="s1sb")
    nc.sync.dma_start(s1sb[:], din["s1_W"][:])
    g1 = [sbp.tile([P, BL], F32, name=f"g1_{j}") for j in range(NP)]
    for mj in range(NP):
        ps1 = psp.tile([P, BL], F32, name=f"ps1{mj}", tag="sm", bufs=2)
        nc.tensor.matmul(ps1[:], s1sb[:, mj * P:(mj + 1) * P], stimT[:],
                         start=True, stop=True)
        nc.scalar.activation(g1[mj][:], ps1[:], AF.Gelu, bias=pv(V_S1B, mj))
    for mj in range(NP):
        ps2 = psp.tile([P, BL], F32, name=f"ps2{mj}", tag="sm", bufs=2)
        for k in range(NP):
            wt = stream_w(din["s2_W"], k, mj, "ws2")
            nc.tensor.matmul(ps2[:], wt, g1[k][:], start=(k == 0), stop=(k == NP - 1))
        nc.scalar.activation(stim_cm[mj][:], ps2[:], AF.Identity, bias=pv(V_S2B, mj))

    def process_ublock(rb, u_blk, step):
        """pooled slice, transpose into u_r, write u_master (or u_out on last step)."""
        for j in range(NP):
            nc.vector.tensor_reduce(pooled[j][:, rb:rb + 1], u_blk[j][:],
                                    mybir.AxisListType.X, OP.add)
        if step < N_STEPS - 1:
            for j in range(NP):
                pst = psp.tile([P, BS], F32, name=f"tp{step}_{rb}_{j}", tag="tp", bufs=2)
                for q in range(BS // P):
                    nc.tensor.transpose(pst[:, q * P:(q + 1) * P],
                                        u_blk[j][:, q * P:(q + 1) * P], ident[:])
                for q in range(BS // P):
                    rc = (rb * BS + q * P) // P
                    nc.scalar.copy(u_r[rc][:, j * P:(j + 1) * P], pst[:, q * P:(q + 1) * P])
                nc.sync.dma_start(u_master[j * P:(j + 1) * P, rb * BS:(rb + 1) * BS],
                                  u_blk[j][:])
        else:
            for j in range(NP):
                nc.sync.dma_start(u_out[rb, j * P:(j + 1) * P, :], u_blk[j][:])

    for rb in range(NB):
        u_blk = []
        for j in range(NP):
            ub = sbw.tile([P, BS], F32, name=f"u0_{rb}_{j}", tag=f"ub{j}", bufs=U_BUFS)
            nc.sync.dma_start(ub[:], din["state_loc"][rb, j * P:(j + 1) * P, :])
            gvb = gv_t[j][:, rb:rb + 1].broadcast_to([P, BS])
            nc.vector.scalar_tensor_tensor(ub[:], ub[:], decay, gvb, OP.mult, OP.add)
            u_blk.append(ub)
        process_ublock(rb, u_blk, step=0)
    for j in range(NP):
        nc.vector.tensor_scalar_mul(pooled[j][:], pooled[j][:], 1.0 / NSP)

    # =========== step loop ===========
    for step in range(N_STEPS):
        # ---- corr from u_r (pre-update u), bf16; scaled by HEB_LR/(B*n) ----
        cc_in, cc_out = cc_in_l[step], cc_out_l[step]
        for mc in range(NP):
            for n4 in range(4):
                psc = psp.tile([P, 512], F32, name=f"cor{step}_{mc}_{n4}", tag="mm", bufs=2)
                for k in range(NRT):
                    nc.tensor.matmul(psc[:], u_r[k][:, mc * P:(mc + 1) * P],
                                     u_r[k][:, n4 * 512:(n4 + 1) * 512],
                                     start=(k == 0), stop=(k == NRT - 1))
                ce = sbw.tile([P, 512], BF16, name=f"ce{step}_{mc}_{n4}", tag="ce", bufs=2)
                nc.scalar.activation(ce[:], psc[:], AF.Copy, scale=lr_eff)
                nc.sync.dma_start(cc_in[mc * P:(mc + 1) * P, n4 * 512:(n4 + 1) * 512], ce[:])
        nc.gpsimd.collective_compute(
            "AllReduce", OP.add, replica_groups=[list(range(NCORES))],
            ins=[cc_in.opt()], outs=[cc_out.opt()])

        # ---- GRU (local BL samples, channel-major) ----
        gi = sbw.tile([P, 48, BL], F32, name=f"gi{step}", tag="gi", bufs=1)
        gh = sbw.tile([P, 48, BL], F32, name=f"gh{step}", tag="gh", bufs=1)
        for (W, rhs_t, out_t, brow) in ((din["gru_Wih"], pooled, gi, 0),
                                        (din["gru_Whh"], h_t, gh, 1)):
            for mj in range(48):
                psg = psp.tile([P, BL], F32, name=f"gru{step}_{brow}_{mj}", tag="sm", bufs=2)
                for k in range(NP):
                    wt = stream_w(W, k, mj, f"wgru{brow}")
                    nc.tensor.matmul(psg[:], wt, rhs_t[k][:], start=(k == 0), stop=(k == NP - 1))
                nc.scalar.activation(out_t[:, mj, :], psg[:], AF.Identity,
                                     bias=gvt[:, brow, mj:mj + 1])
        for j in range(NP):
            rt = sbw.tile([P, BL], F32, name=f"rt{step}_{j}", tag="rt", bufs=2)
            zt = sbw.tile([P, BL], F32, name=f"zt{step}_{j}", tag="zt", bufs=2)
            nt = sbw.tile([P, BL], F32, name=f"nt{step}_{j}", tag="nt", bufs=2)
            tmp = sbw.tile([P, BL], F32, name=f"gtmp{step}_{j}", tag="gtmp", bufs=2)
            nc.vector.tensor_tensor(tmp[:], gi[:, j, :], gh[:, j, :], OP.add)
            nc.scalar.activation(rt[:], tmp[:], AF.Sigmoid)
            nc.vector.tensor_tensor(tmp[:], gi[:, 16 + j, :], gh[:, 16 + j, :], OP.add)
            nc.scalar.activation(zt[:], tmp[:], AF.Sigmoid)
            nc.vector.tensor_tensor(tmp[:], gh[:, 32 + j, :], rt[:], OP.mult)
            nc.vector.tensor_tensor(tmp[:], tmp[:], gi[:, 32 + j, :], OP.add)
            nc.scalar.activation(nt[:], tmp[:], AF.Tanh)
            nc.vector.tensor_tensor(tmp[:], h_t[j][:], nt[:], OP.subtract)
            nc.vector.tensor_tensor(tmp[:], tmp[:], zt[:], OP.mult)
            nc.vector.tensor_tensor(h_t[j][:], tmp[:], nt[:], OP.add)
            nc.vector.scalar_tensor_tensor(sbdt[j][:], h_t[j][:], ap_sp, stim_cm[j][:],
                                           OP.mult, OP.add)
            nc.vector.tensor_scalar_mul(sbdt[j][:], sbdt[j][:], dt)

        # ---- Wh update: wh_mm = HEB_DECAY*wh_old + corr_ar ----
        wh_src = din["heb_W"] if step == 0 else wh_mm
        for j in range(NP):
            for cb in range(4):
                r0, c0 = j * P, cb * 512
                who = sbw.tile([P, 512], F32, name=f"who{step}_{j}_{cb}", tag="who", bufs=2)
                nc.sync.dma_start(who[:], wh_src[r0:r0 + P, c0:c0 + 512])
                crt = sbw.tile([P, 512], BF16, name=f"crt{step}_{j}_{cb}", tag="crt", bufs=2)
                nc.sync.dma_start(crt[:], cc_out[r0:r0 + P, c0:c0 + 512])
                nc.vector.scalar_tensor_tensor(who[:], who[:], HEB_DECAY, crt[:],
                                               OP.mult, OP.add)
                nc.sync.dma_start(wh_mm[r0:r0 + P, c0:c0 + 512], who[:])

        # ---- row blocks ----
        for rb in range(NB):
            u_blk, diff_blk = [], []
            for j in range(NP):
                ub = sbw.tile([P, BS], F32, name=f"u{step}_{rb}_{j}", tag=f"ub{j}", bufs=U_BUFS)
                nc.sync.dma_start(ub[:], u_master[j * P:(j + 1) * P, rb * BS:(rb + 1) * BS])
                u_blk.append(ub)
            for j in range(NP):
                db = sbw.tile([P, BS], F32, name=f"d{step}_{rb}_{j}", tag=f"db{j}", bufs=1)
                u3 = u_blk[j][:].rearrange("p (h w) -> p h w", h=HW)
                d3 = db[:].rearrange("p (h w) -> p h w", h=HW)
                nc.vector.tensor_scalar_mul(db[:], u_blk[j][:], pv(V_CSN4, j))
                for di, d in enumerate(DILS):
                    cf = pv(V_CA, j) if di == 0 else pv(V_CB, j)
                    nc.vector.scalar_tensor_tensor(d3[:, :, d:], u3[:, :, :HW - d], cf,
                                                   d3[:, :, d:], OP.mult, OP.add)
                    nc.vector.scalar_tensor_tensor(d3[:, :, :HW - d], u3[:, :, d:], cf,
                                                   d3[:, :, :HW - d], OP.mult, OP.add)
                    nc.vector.scalar_tensor_tensor(d3[:, d:, :], u3[:, :HW - d, :], cf,
                                                   d3[:, d:, :], OP.mult, OP.add)
                    nc.vector.scalar_tensor_tensor(d3[:, :HW - d, :], u3[:, d:, :], cf,
                                                   d3[:, :HW - d, :], OP.mult, OP.add)
                diff_blk.append(db)
            act1 = []
            for mj in range(NP):
                ps = psp.tile([P, BS], F32, name=f"m1_{step}_{rb}_{mj}", tag="mm", bufs=2)
                for k in range(NP):
                    wt = stream_w(din["r1_W"], k, mj, "w1")
                    nc.tensor.matmul(ps[:], wt, u_blk[k][:], start=(k == 0), stop=(k == NP - 1))
                a1 = sbw.tile([P, BS], F32, name=f"a1_{step}_{rb}_{mj}", tag=f"a1{mj}", bufs=1)
                nc.scalar.activation(a1[:], ps[:], AF.Gelu, bias=pv(V_R1B, mj))
                act1.append(a1)
            for mj in range(NP):
                ps = psp.tile([P, BS], F32, name=f"m2_{step}_{rb}_{mj}", tag="mm", bufs=2)
                for k in range(NP):
                    wt = stream_w(din["r2_W"], k, mj, "w2")
                    nc.tensor.matmul(ps[:], wt, act1[k][:], start=(k == 0), stop=False)
                for k in range(NP):
                    wt = stream_w(wh_mm, k, mj, "wh")
                    nc.tensor.matmul(ps[:], wt, diff_blk[k][:], start=False, stop=(k == NP - 1))
                du = sbw.tile([P, BS], F32, name=f"du{step}_{rb}_{mj}", tag="du", bufs=2)
                nc.scalar.activation(du[:], ps[:], AF.Identity, bias=pv(V_R2B, mj))
                nc.vector.tensor_tensor(du[:], du[:], diff_blk[mj][:], OP.add)
                nc.vector.scalar_tensor_tensor(u_blk[mj][:], du[:], dt, u_blk[mj][:],
                                               OP.mult, OP.add)
                sbb = sbdt[mj][:, rb:rb + 1].broadcast_to([P, BS])
                nc.vector.tensor_tensor(u_blk[mj][:], u_blk[mj][:], sbb, OP.add)
            if (step + 1) % 2 == 0:
                ssq = psp.tile([1, BS], F32, name=f"ssq{step}_{rb}", tag="tp", bufs=2)
                for j in range(NP):
                    u2 = sbw.tile([P, BS], F32, name=f"u2_{step}_{rb}_{j}", tag="u2", bufs=2)
                    nc.scalar.square(u2[:], u_blk[j][:])
                    nc.tensor.matmul(ssq[:], ones[:], u2[:], start=(j == 0), stop=(j == NP - 1))
                rms = sbw.tile([1, BS], F32, name=f"rms{step}_{rb}", tag="rms", bufs=2)
                nc.scalar.activation(rms[:], ssq[:], AF.Sqrt, bias=eps6[0:1, :], scale=1.0 / C)
                nc.vector.reciprocal(rms[:], rms[:])
                rmsb = sbw.tile([P, BS], F32, name=f"rmsb{step}_{rb}", tag="rmsb", bufs=2)
                nc.gpsimd.partition_broadcast(rmsb[:], rms[:])
                for j in range(NP):
                    nc.vector.tensor_tensor(u_blk[j][:], u_blk[j][:], rmsb[:], OP.mult)
                    nc.vector.tensor_scalar_mul(u_blk[j][:], u_blk[j][:], pv(V_RMSW, j))
            process_ublock(rb, u_blk, step)
        for j in range(NP):
            nc.vector.tensor_scalar_mul(pooled[j][:], pooled[j][:], 1.0 / NSP)

    # =========== read phase: rep = LN(pooled) @ read_W + read_b ===========
    p_r = sbp.tile([BL, C], F32, name="p_r")
    for j in range(NP):
        pt8 = psp.tile([BL, P], F32, name=f"pt8_{j}", tag="tp8", bufs=1)
        nc.tensor.transpose(pt8[:], pooled[j][:], ident[:])
        nc.scalar.copy(p_r[:, j * P:(j + 1) * P], pt8[:])
    mu = sbp.tile([BL, 1], F32, name="mu")
    nc.vector.tensor_reduce(mu[:], p_r[:], mybir.AxisListType.X, OP.add)
    nc.vector.tensor_scalar_mul(mu[:], mu[:], 1.0 / C)
    nc.vector.tensor_scalar(p_r[:], p_r[:], mu[:], None, OP.subtract)
    nc.vector.tensor_tensor(p_r[:], p_r[:], p_r[:], OP.mult)
    var = sbp.tile([BL, 1], F32, name="var")
    nc.vector.tensor_reduce(var[:], p_r[:], mybir.AxisListType.X, OP.add)
    rstd = sbp.tile([BL, 1], F32, name="rstd")
    nc.scalar.activation(rstd[:], var[:], AF.Sqrt, bias=eps5[0:BL, :], scale=1.0 / C)
    nc.vector.reciprocal(rstd[:], rstd[:])
    muT = psp.tile([1, BL], F32, name="muT", tag="tp8", bufs=1)
    nc.tensor.transpose(muT[:], mu[:], ident[:BL, :BL])
    muS = sbp.tile([1, BL], F32, name="muS")
    nc.scalar.copy(muS[:], muT[:])
    rsT = psp.tile([1, BL], F32, name="rsT", tag="tp8", bufs=1)
    nc.tensor.transpose(rsT[:], rstd[:], ident[:BL, :BL])
    rsS = sbp.tile([1, BL], F32, name="rsS")
    nc.scalar.copy(rsS[:], rsT[:])
    mub = sbp.tile([P, BL], F32, name="mub")
    nc.gpsimd.partition_broadcast(mub[:], muS[:])
    rstdb = sbp.tile([P, BL], F32, name="rstdb")
    nc.gpsimd.partition_broadcast(rstdb[:], rsS[:])
    lnp = [sbp.tile([P, BL], F32, name=f"lnp{j}") for j in range(NP)]
    for j in range(NP):
        nc.vector.tensor_tensor(lnp[j][:], pooled[j][:], mub[:], OP.subtract)
        nc.vector.tensor_tensor(lnp[j][:], lnp[j][:], rstdb[:], OP.mult)
        nc.vector.tensor_scalar(lnp[j][:], lnp[j][:], pv(V_LNG, j), pv(V_LNB, j),
                                OP.mult, OP.add)
    rep_r = sbp.tile([BL, C], F32, name="rep_r")
    for mj in range(NP):
        psr = psp.tile([P, BL], F32, name=f"rd{mj}", tag="sm", bufs=2)
        for k in range(NP):
            wt = stream_w(din["read_W"], k, mj, "wrd")
            nc.tensor.matmul(psr[:], wt, lnp[k][:], start=(k == 0), stop=(k == NP - 1))
        rc = sbw.tile([P, BL], F32, name=f"rc{mj}", tag="rc", bufs=2)
        nc.scalar.activation(rc[:], psr[:], AF.Identity, bias=pv(V_READB, mj))
        rt8 = psp.tile([BL, P], F32, name=f"rt8_{mj}", tag="tp8", bufs=1)
        nc.tensor.transpose(rt8[:], rc[:], ident[:])
        nc.scalar.copy(rep_r[:, mj * P:(mj + 1) * P], rt8[:])
    nc.sync.dma_start(rep_out[:], rep_r[:])

    for pool in (psp, sbw, sbp, dram):
        pool.release()


_compiled = None
_compiled_sc = None
_last_in_maps = None


def _build_program(sc):
    nc = bacc.Bacc("TRN2", target_bir_lowering=False, debug=False, num_devices=NCORES)
    din, u_out, rep_out = declare_io(nc)
    with tile.TileContext(nc) as tc:
        emit(nc, tc, din, u_out, rep_out, sc)
    nc.compile()
    return nc


def kernel(**inputs):
    global _compiled, _compiled_sc
    inp = {k: np.asarray(v) for k, v in inputs.items()}
    decay = float(np.clip(np.exp(inp["log_decay"]), 0.5, 0.99))
    dt = float(np.clip(np.exp(inp["log_dt"]), 0.01, 0.3))
    ap_sp = float(np.log1p(np.exp(inp["alpha_pump"])))
    sc = {"decay": decay, "dt": dt, "ap_sp": ap_sp}

    if _compiled is None or _compiled_sc != sc:
        _compiled = _build_program(sc)
        _compiled_sc = sc
    nc = _compiled

    coeff = (DIFF_SCALE / (1.0 + np.exp(-inp["diff_alpha"].astype(np.float64)))).astype(np.float32)
    csn4 = (-4.0 * coeff.sum(0)).astype(np.float32)
    vecs = np.stack([
        inp["gate_b"], inp["value_b"], inp["r1_b"], inp["r2_b"], inp["rms_w"],
        inp["ln_g"], inp["ln_b"], inp["read_b"], inp["s2_b"], coeff[0], coeff[1],
        csn4, inp["s1_b"],
    ]).astype(np.float32)
    gvecs = np.stack([inp["gru_bih"], inp["gru_bhh"]]).astype(np.float32)

    shared = {
        "heb_W": inp["heb_W"], "gate_W": inp["gate_W"], "value_W": inp["value_W"],
        "r1_W": inp["r1_W"], "r2_W": inp["r2_W"], "read_W": inp["read_W"],
        "s2_W": inp["s2_W"], "gru_Wih": inp["gru_Wih"], "gru_Whh": inp["gru_Whh"],
        "s1_W": inp["s1_W"], "vecs": vecs, "gvecs": gvecs,
    }
    shared = {k: np.ascontiguousarray(v, dtype=np.float32) for k, v in shared.items()}

    state = inp["state"].reshape(B, C, NSP)
    in_maps = []
    for i in range(NCORES):
        b0 = i * BL
        m = dict(shared)
        m["state_loc"] = np.ascontiguousarray(state[b0:b0 + BL], np.float32)
        m["obs_loc"] = np.ascontiguousarray(inp["observation"][b0:b0 + BL], np.float32)
        m["stim_loc"] = np.ascontiguousarray(inp["stimulus"][b0:b0 + BL], np.float32)
        m["h_loc"] = np.ascontiguousarray(inp["h_global"][b0:b0 + BL], np.float32)
        in_maps.append(m)

    global _last_in_maps
    _last_in_maps = in_maps
    res = run_bass_kernel_spmd(nc, in_maps, list(range(NCORES))).results
    u_full = np.concatenate([r["u_out"] for r in res], axis=0).reshape(B, C, HW, HW)
    rep_full = np.concatenate([r["rep_out"] for r in res], axis=0)
    return (u_full.astype(np.float32), rep_full.astype(np.float32))
